# revision 13
# baseline (speedup 1.0000x reference)
"""Trainium2 Bass kernel for packed-sequence GRU decoder (nn_Decoder).

Reference semantics (T=512, B=1024, V=64, H=100):
  per step t: h = where(t < len, GRUCell(x_t, h), h)
              out_t = where(t < len, log_softmax(h @ W_out.T + b_out), 0)

Architecture (v2):
  - Data-parallel over batch, STRIDED: core k owns lanes k, k+8, ... (128).
    lengths sorted descending => striding load-balances; each core's active
    lanes are a prefix at every step.  lengths specialized into the program
    as static per-step active-lane counts (sched).
  - All-tanh formulation: sigmoid(x) = (tanh(x/2)+1)/2, so the recurrence
    needs only Tanh; the softmax needs only Exp.  Both live in the single
    ACT table set 'exp_and_others', so the ACT engine never reloads tables.
    Ln (for log-softmax) is replaced by an exponent/mantissa bit-trick
    log computed on the DVE with a small polynomial.
  - bf16 everywhere on the PE (4x fp32 matmul throughput); PSUM stays fp32.
  - Linearity split: h' = p + s with s = z*h (ready early, right after
    sig_z) and p = (1-z)*n (critical, after tanh).  The next step's gate
    matmuls consume s and p SEPARATELY (W h' = W s + W p), so the h-add
    leaves the critical path entirely; h is carried as hh = 0.5*h (the
    halves fall out of the tanh-sigmoid identity) and only used off-chain.
  - Critical chain per step: p -> [W_r p matmul] -> tanh(.5 g_r) -> u ->
    v -> tanh(v) -> p, with u/v/p as scalar_tensor_tensor ops in bf16
    (4x DVE mode).  Everything else (sig_z pool work, logits matmul,
    exp+accum softmax reduction, lnS poly, output assembly, DMA) hides in
    engine idle slots.
  - Softmax: per-step Exp with accum_out accumulates S = sum_v exp(logit)
    on the fly; at chunk boundaries lnS is computed on DVE (bit-trick log)
    and out = logits - lnS is assembled and DMA'd.  Host zero-fills padded
    positions.
"""

import numpy as np

T, B, V, H = 512, 1024, 64, 100
NCORES = 8
BL = B // NCORES          # 128 lanes per core
KX = V + 1                # 65: x rows + ones row
KH = H + 1                # 101: h rows + const row
TC = 16                   # timesteps per softmax chunk (PSUM-limited)

# ln(1+t) ~= t*(C1 + t*(C2 + t*(C3 + t*C4))) on [0,1), max abs err 5.1e-4
LN_C = (0.9993014659280711, -0.4846364440964482,
        0.25187601351892186, -0.0738994060884228)
LN2 = 0.6931471805599453

_prog_cache: dict = {}


def _build(sched, t_steps, tc_steps):
    import concourse.bass as bass
    import concourse.mybir as mybir
    from concourse import bacc, tile
    from concourse.tile_rust import add_dep_helper

    f32 = mybir.dt.float32
    bf16 = mybir.dt.bfloat16
    i32 = mybir.dt.int32
    AF = mybir.ActivationFunctionType
    ALU = mybir.AluOpType

    nc = bacc.Bacc()

    xT = nc.declare_dram_parameter("xT", [t_steps, KX, BL], bf16, isOutput=False)
    HH0 = nc.declare_dram_parameter("hh0", [KH, BL], bf16, isOutput=False)
    WX = nc.declare_dram_parameter("WX", [KX, 3 * H], bf16, isOutput=False)
    WHS = nc.declare_dram_parameter("WHS", [KH, 3 * H], bf16, isOutput=False)
    WH2 = nc.declare_dram_parameter("WH2", [KH, 3 * H], bf16, isOutput=False)
    WO2 = nc.declare_dram_parameter("WO2", [KH, V], bf16, isOutput=False)
    ONES = nc.declare_dram_parameter("ones", [1, BL], bf16, isOutput=False)
    OUT = nc.declare_dram_parameter("out", [t_steps, BL, V], f32, isOutput=True)

    xTr = xT.rearrange("t p l -> p t l")
    OUTr = OUT.rearrange("t l v -> l t v")

    n_chunks = t_steps // tc_steps

    with tile.TileContext(nc) as tc:
        with (
            tc.tile_pool(name="const", bufs=1) as cpool,
            tc.tile_pool(name="xin", bufs=3) as xpool,
            tc.tile_pool(name="work", bufs=2) as wpool,
            tc.tile_pool(name="soft", bufs=2) as spool,
            tc.tile_pool(name="pgate", bufs=1, space="PSUM") as pgp,
            tc.tile_pool(name="plgp", bufs=2, space="PSUM") as plgp,
        ):
            wx = cpool.tile([KX, 3 * H], bf16)
            whs = cpool.tile([KH, 3 * H], bf16)
            wh2 = cpool.tile([KH, 3 * H], bf16)
            wo2 = cpool.tile([KH, V], bf16)
            hh = cpool.tile([KH, BL], bf16)
            nc.sync.dma_start(wx[:], WX[:])
            nc.sync.dma_start(whs[:], WHS[:])
            nc.sync.dma_start(wh2[:], WH2[:])
            nc.sync.dma_start(wo2[:], WO2[:])
            nc.sync.dma_start(hh[:], HH0[:])
            # s tile: rows 0..H-1 written per step; row H is the constant 1.0
            # that routes b_hn (in WHS row H) into the n-gate h-part.
            s_aug = cpool.tile([KH, BL], bf16)
            nc.sync.dma_start(s_aug[H : H + 1, :], ONES[:])

            def load_chunk(c):
                t0 = c * tc_steps
                ncm = sched[t0]
                xb = xpool.tile([KX, tc_steps, BL], bf16, tag="xb")
                nc.sync.dma_start(xb[:, :, 0:ncm], xTr[:, t0 : t0 + tc_steps, 0:ncm])
                return xb

            PB = 512  # one PSUM bank in fp32 elems; one accumulation group per bank
            R_R, R_Z, R_NB, R_NA = 0 * PB, 1 * PB, 2 * PB, 3 * PB

            def emit_x_mms(t, xbuf, pgt):
                # x-part matmuls for step t's gates (start each PSUM group)
                n_ = sched[t]
                xt_ = xbuf[:, t % tc_steps, 0:n_]
                nc.tensor.matmul(pgt[:, R_R : R_R + n_], wx[:, 0:H], xt_,
                                 start=True, stop=False)
                nc.tensor.matmul(pgt[:, R_Z : R_Z + n_], wx[:, H : 2 * H], xt_,
                                 start=True, stop=False)
                # n-gate x-part has its own bank (read directly by vpre)
                nc.tensor.matmul(pgt[:, R_NA : R_NA + n_], wx[:, 2 * H : 3 * H], xt_,
                                 start=True, stop=True)

            def emit_s_mms(t, pgt, rhs):
                # h-part from s (= z*h_prev); n-gate lhsT includes b_hn row
                n_ = sched[t]
                r_ = rhs[0:H, 0:n_]
                nc.tensor.matmul(pgt[:, R_R : R_R + n_], whs[0:H, 0:H], r_,
                                 start=False, stop=False)
                nc.tensor.matmul(pgt[:, R_Z : R_Z + n_], whs[0:H, H : 2 * H], r_,
                                 start=False, stop=False)
                nc.tensor.matmul(pgt[:, R_NB : R_NB + n_], whs[:, 2 * H : 3 * H],
                                 rhs[:, 0:n_], start=True, stop=False)

            def emit_p_mms(t, pgt, rhs):
                # h-part from p (= (1-z)*n); closes each group. r-gate first.
                n_ = sched[t]
                r_ = rhs[0:H, 0:n_]
                nc.tensor.matmul(pgt[:, R_R : R_R + n_], whs[0:H, 0:H], r_,
                                 start=False, stop=True)
                nc.tensor.matmul(pgt[:, R_Z : R_Z + n_], whs[0:H, H : 2 * H], r_,
                                 start=False, stop=True)
                nc.tensor.matmul(pgt[:, R_NB : R_NB + n_], whs[0:H, 2 * H : 3 * H], r_,
                                 start=False, stop=True)

            def emit_h0_mms(pgt):
                # prologue: gates(0) h-part directly from hh0 (doubled weights)
                n_ = sched[0]
                r_ = hh[:, 0:n_]
                nc.tensor.matmul(pgt[:, R_R : R_R + n_], wh2[:, 0:H], r_,
                                 start=False, stop=True)
                nc.tensor.matmul(pgt[:, R_Z : R_Z + n_], wh2[:, H : 2 * H], r_,
                                 start=False, stop=True)
                nc.tensor.matmul(pgt[:, R_NB : R_NB + n_], wh2[:, 2 * H : 3 * H], r_,
                                 start=True, stop=True)

            def emit_lnS(S, ncm):
                # lnS = ln(S) via exponent/mantissa split + deg-3 poly (DVE)
                C1, C2, C3, C4 = LN_C
                iv = S[0:ncm].bitcast(i32)
                eI = spool.tile([BL, tc_steps], i32, tag="eI")
                nc.vector.tensor_scalar(eI[0:ncm], iv, 23, None,
                                        ALU.logical_shift_right)
                eF = spool.tile([BL, tc_steps], f32, tag="eF")
                nc.vector.tensor_copy(eF[0:ncm], eI[0:ncm])
                # eF <- (e_raw - 127) * ln2
                nc.vector.tensor_scalar(eF[0:ncm], eF[0:ncm], 127.0, LN2,
                                        ALU.subtract, ALU.mult)
                mI = spool.tile([BL, tc_steps], i32, tag="mI")
                nc.vector.tensor_scalar(mI[0:ncm], iv, 0x007FFFFF, 0x3F800000,
                                        ALU.bitwise_and, ALU.bitwise_or)
                t1 = spool.tile([BL, tc_steps], f32, tag="t1")
                nc.vector.tensor_scalar(t1[0:ncm], mI[0:ncm].bitcast(f32), 1.0, None,
                                        ALU.subtract)
                q = spool.tile([BL, tc_steps], f32, tag="q")
                nc.vector.tensor_scalar(q[0:ncm], t1[0:ncm], C4, C3, ALU.mult, ALU.add)
                nc.vector.scalar_tensor_tensor(q[0:ncm], q[0:ncm], 0.0, t1[0:ncm],
                                               ALU.bypass, ALU.mult)
                nc.vector.tensor_scalar(q[0:ncm], q[0:ncm], C2, None, ALU.add)
                nc.vector.scalar_tensor_tensor(q[0:ncm], q[0:ncm], 0.0, t1[0:ncm],
                                               ALU.bypass, ALU.mult)
                nc.vector.tensor_scalar(q[0:ncm], q[0:ncm], C1, None, ALU.add)
                nc.vector.scalar_tensor_tensor(q[0:ncm], q[0:ncm], 0.0, t1[0:ncm],
                                               ALU.bypass, ALU.mult)
                lnS = spool.tile([BL, tc_steps], f32, tag="lnS")
                nc.vector.scalar_tensor_tensor(lnS[0:ncm], eF[0:ncm], 0.0, q[0:ncm],
                                               ALU.bypass, ALU.add)
                return lnS

            # ---- prologue: group 0 ----
            xb_cur = load_chunk(0)
            xb_next = load_chunk(1) if n_chunks > 1 else None
            pgt_cur = pgp.tile([H, 4 * 512], f32, tag="pg")
            emit_x_mms(0, xb_cur, pgt_cur)
            emit_h0_mms(pgt_cur)

            plg = None
            plg_prev = None
            S_cur = None
            S_prev = None
            lnS_prev = None
            ob_prev = None
            ncm_prev = None
            t0_prev = None

            for t in range(t_steps):
                c, tl = divmod(t, tc_steps)
                n = sched[t]
                if tl == 0:
                    t0 = t
                    ncm = sched[t0]
                    plg_prev_ = plg
                    plg = plgp.tile([BL, tc_steps, V], f32, tag="plg", name="plg")
                    S_prev_ = S_cur
                    S_cur = spool.tile([BL, tc_steps], f32, tag="S", name="S")
                    if t > 0:
                        plg_prev = plg_prev_
                        S_prev = S_prev_
                        xb_cur = xb_next
                        xb_next = load_chunk(c + 1) if c + 1 < n_chunks else None

                pgt = pgt_cur

                # --- x-part matmuls for step t+1 (next psum buffer) ---
                if t + 1 < t_steps:
                    pgt_next = pgp.tile([H, 4 * 512], f32, tag="pg")
                    nxt_buf = xb_cur if (t + 1) // tc_steps == c else xb_next
                    emit_x_mms(t + 1, nxt_buf, pgt_next)

                # --- critical: sig_r as tanh(g_r/2) ---
                rz = wpool.tile([H, 2 * BL], bf16, tag="rz")
                sig_r = nc.scalar.activation(rz[:, 0:n], pgt[:, R_R : R_R + n],
                                             AF.Tanh, scale=0.5)
                sig_z = nc.scalar.activation(rz[:, BL : BL + n],
                                             pgt[:, R_Z : R_Z + n], AF.Tanh, scale=0.5)
                add_dep_helper(sig_z.ins, sig_r.ins, reason="sig_r unblocks u first")

                # --- DVE prep during sig_r: w = 0.5*phn, vpre = w + i_n ---
                w = wpool.tile([H, BL], bf16, tag="w")
                nc.vector.tensor_scalar(w[:, 0:n], pgt[:, R_NB : R_NB + n],
                                        0.5, None, ALU.mult)
                vpre = wpool.tile([H, BL], bf16, tag="vpre")
                nc.vector.scalar_tensor_tensor(vpre[:, 0:n], w[:, 0:n], 0.0,
                                               pgt[:, R_NA : R_NA + n],
                                               ALU.bypass, ALU.add)

                # --- critical: u = tanh_r*w ; v = u + vpre (bf16, 4x mode) ---
                u = wpool.tile([H, BL], bf16, tag="u")
                nc.vector.scalar_tensor_tensor(u[:, 0:n], rz[:, 0:n], 0.0,
                                               w[:, 0:n], ALU.bypass, ALU.mult)
                v = wpool.tile([H, BL], bf16, tag="v")
                nc.vector.scalar_tensor_tensor(v[:, 0:n], u[:, 0:n], 0.0,
                                               vpre[:, 0:n], ALU.bypass, ALU.add)

                # --- critical: n_t = tanh(v) ---
                nt = wpool.tile([H, BL], bf16, tag="nt")
                nt_act = nc.scalar.activation(nt[:, 0:n], v[:, 0:n], AF.Tanh)

                # --- Pool (off-chain, after sig_z): zb, q, s, sh ---
                zb = wpool.tile([H, BL], bf16, tag="zb")
                nc.gpsimd.tensor_scalar(zb[:, 0:n], rz[:, BL : BL + n], -0.5, 0.5,
                                        ALU.mult, ALU.add)
                qh = wpool.tile([H, BL], bf16, tag="qh")
                nc.gpsimd.tensor_mul(qh[:, 0:n], rz[:, BL : BL + n], hh[0:H, 0:n])
                nc.gpsimd.tensor_add(s_aug[0:H, 0:n], qh[:, 0:n], hh[0:H, 0:n])
                sh = wpool.tile([H, BL], bf16, tag="sh")
                nc.gpsimd.tensor_scalar(sh[:, 0:n], s_aug[0:H, 0:n], 0.5, None, ALU.mult)

                # --- s-part matmuls for step t+1 ---
                if t + 1 < t_steps:
                    emit_s_mms(t + 1, pgt_next, s_aug)

                # --- critical: p = zb * n_t ; then p-part matmuls t+1 ---
                p = wpool.tile([H, BL], bf16, tag="p")
                nc.vector.scalar_tensor_tensor(p[:, 0:n], zb[:, 0:n], 0.0,
                                               nt[:, 0:n], ALU.bypass, ALU.mult)
                if t + 1 < t_steps:
                    emit_p_mms(t + 1, pgt_next, p)

                # --- hh update (off-chain): hh = 0.5*p + sh ---
                nc.vector.scalar_tensor_tensor(hh[0:H, 0:n], p[:, 0:n], 0.5,
                                               sh[:, 0:n], ALU.mult, ALU.add)

                # --- logits for step t (chunk-max lanes) ---
                nc.tensor.matmul(plg[0:ncm, tl, :], hh[:, 0:ncm], wo2[:],
                                 start=True, stop=True)

                # --- softmax bookkeeping for step t-1 (prev logits ready) ---
                if t > 0:
                    tp = t - 1
                    cp_, tlp = divmod(tp, tc_steps)
                    plg_p = plg if cp_ == c else plg_prev
                    ncm_p = sched[cp_ * tc_steps]
                    S_p = S_cur if cp_ == c else S_prev
                    E = wpool.tile([BL, V], bf16, tag="E")
                    nc.scalar.activation(E[0:ncm_p], plg_p[0:ncm_p, tlp, :], AF.Exp,
                                         accum_out=S_p[0:ncm_p, tlp : tlp + 1])

                # --- previous chunk tail work, spread across early bodies ---
                if t > 0 and 1 <= tl <= 3 and c > 0:
                    pc = c - 1
                    if tl == 1:
                        ncm_prev = sched[pc * tc_steps]
                        t0_prev = pc * tc_steps
                        lnS_prev = emit_lnS(S_prev, ncm_prev)
                        ob_prev = spool.tile([BL, tc_steps, V], f32, tag="ob")
                    elif tl == 2:
                        hc = tc_steps // 2
                        nc.vector.scalar_tensor_tensor(
                            ob_prev[0:ncm_prev, 0:hc], plg_prev[0:ncm_prev, 0:hc], 0.0,
                            lnS_prev[0:ncm_prev, 0:hc].broadcast_to(
                                [ncm_prev, hc, V]),
                            ALU.bypass, ALU.subtract)
                        nc.vector.scalar_tensor_tensor(
                            ob_prev[0:ncm_prev, hc:tc_steps],
                            plg_prev[0:ncm_prev, hc:tc_steps], 0.0,
                            lnS_prev[0:ncm_prev, hc:tc_steps].broadcast_to(
                                [ncm_prev, tc_steps - hc, V]),
                            ALU.bypass, ALU.subtract)
                    elif tl == 3:
                        nc.sync.dma_start(
                            OUTr[0:ncm_prev, t0_prev : t0_prev + tc_steps, :],
                            ob_prev[0:ncm_prev])

                if t + 1 < t_steps:
                    pgt_cur = pgt_next

            # ---- epilogue: last step's exp + last chunk's output ----
            tl_last = (t_steps - 1) % tc_steps
            ncm_l = sched[(n_chunks - 1) * tc_steps]
            E = wpool.tile([BL, V], bf16, tag="E")
            nc.scalar.activation(E[0:ncm_l], plg[0:ncm_l, tl_last, :], AF.Exp,
                                 accum_out=S_cur[0:ncm_l, tl_last : tl_last + 1])
            lnS_l = emit_lnS(S_cur, ncm_l)
            ob_l = spool.tile([BL, tc_steps, V], f32, tag="ob")
            nc.vector.scalar_tensor_tensor(
                ob_l[0:ncm_l], plg[0:ncm_l], 0.0,
                lnS_l[0:ncm_l].broadcast_to([ncm_l, tc_steps, V]),
                ALU.bypass, ALU.subtract)
            t0_l = (n_chunks - 1) * tc_steps
            nc.sync.dma_start(OUTr[0:ncm_l, t0_l : t0_l + tc_steps, :], ob_l[0:ncm_l])

    nc.compile()
    return nc


def _schedule(lengths, t_steps):
    counts = (np.asarray(lengths)[None, :] > np.arange(t_steps)[:, None]).sum(axis=1)
    return tuple(max(1, int(-(-int(c) // NCORES))) for c in counts)


def _np_bf16():
    import ml_dtypes
    return ml_dtypes.bfloat16


def _prepare(inputs):
    bf = _np_bf16()
    x = np.ascontiguousarray(np.asarray(inputs["x"], dtype=np.float32))
    h0 = np.asarray(inputs["h"], dtype=np.float32)
    lengths = np.asarray(inputs["lengths"], dtype=np.int32)
    W_ih = np.asarray(inputs["W_ih"], dtype=np.float32)
    W_hh = np.asarray(inputs["W_hh"], dtype=np.float32)
    b_ih = np.asarray(inputs["b_ih"], dtype=np.float32)
    b_hh = np.asarray(inputs["b_hh"], dtype=np.float32)
    W_out = np.asarray(inputs["W_out"], dtype=np.float32)
    b_out = np.asarray(inputs["b_out"], dtype=np.float32)

    sched = _schedule(lengths, T)
    key = (sched, T, TC)
    if key not in _prog_cache:
        _prog_cache[key] = _build(sched, T, TC)
    nc = _prog_cache[key]

    # WX: x-part weights; bias row carries b_ih for all gates plus b_hh for
    # the r,z gates (additive); b_hn instead rides WHS's row H (inside r*(.)).
    WXh = np.empty((KX, 3 * H), np.float32)
    WXh[:V] = W_ih.T
    WXh[V] = b_ih
    WXh[V, 0 : 2 * H] += b_hh[0 : 2 * H]
    WHSh = np.zeros((KH, 3 * H), np.float32)
    WHSh[:H] = W_hh.T
    WHSh[H, 2 * H : 3 * H] = b_hh[2 * H : 3 * H]
    WH2h = np.empty((KH, 3 * H), np.float32)
    WH2h[:H] = 2.0 * W_hh.T
    WH2h[H] = 2.0 * b_hh
    WO2h = np.empty((KH, V), np.float32)
    WO2h[:H] = 2.0 * W_out.T
    WO2h[H] = 2.0 * b_out

    WXb = WXh.astype(bf)
    WHSb = WHSh.astype(bf)
    WH2b = WH2h.astype(bf)
    WO2b = WO2h.astype(bf)

    in_maps = []
    for k in range(NCORES):
        xs = x[:, k::NCORES, :]  # [T, BL, V]
        xTk = np.empty((T, KX, BL), bf)
        xTk[:, :V, :] = xs.transpose(0, 2, 1).astype(bf)
        xTk[:, V, :] = 1.0
        hhk = np.empty((KH, BL), bf)
        hhk[:H] = (0.5 * h0[0, k::NCORES, :].T).astype(bf)
        hhk[H] = 0.5
        in_maps.append({"xT": xTk, "hh0": hhk, "WX": WXb, "WHS": WHSb,
                        "WH2": WH2b, "WO2": WO2b,
                        "ones": np.ones((1, BL), bf)})

    return nc, in_maps, lengths


def kernel(**inputs):
    nc, in_maps, lengths = _prepare(inputs)

    from concourse.bass_utils import run_bass_kernel_spmd

    res = run_bass_kernel_spmd(nc, in_maps, list(range(NCORES))).results

    full = np.zeros((T, B, V), dtype=np.float32)
    for k in range(NCORES):
        full[:, k::NCORES, :] = res[k]["out"]
    full[np.arange(T)[:, None] >= lengths[None, :]] = 0.0
    return full
